# revision 20
# baseline (speedup 1.0000x reference)
"""Cross-Spatial-Attention Trainium2 kernel (8 NeuronCores, spatial sharding).

Strategy: shard the 256-row image into 8 bands of 32 rows (both batch elements
on every core, 1-row halos for the 3x3 depthwise convs). All convolutions and
the attention application are then fully local; the only cross-core data is the
channel-gram / norm / mean statistics (two small AllReduces).

Key formulations:
  - dwconv3x3(conv1x1(x)) == sum over 9 taps of shifted matmuls with
    per-tap-combined weights (PSUM accumulation) for the qk and v branches.
  - q,k are produced directly transposed ([n,c] layout) via
    out_chunk = x_chunk^T @ W_tap^T, so the channel gram needs no transpose
    pass and the spatial gate `sa` is a per-partition scalar.
  - the y-branch depthwise conv runs on the Vector engine (9 shifted
    multiply-accumulates in bf16 2x mode; odd-column taps read a host-shifted
    copy of y to keep 4-byte alignment), freeing ~75us of PE time.
  - stats AllReduce is split: gram/norms reduce right after the qk convs and
    is hidden by the v convs; the tiny v-mean reduce follows and is hidden by
    the softmax/Meff math.
  - softmax over a full 128x128 gram with a block-diagonal mask; the
    attention apply + output projection collapse into one matmul
    (Meff = proj @ attnBD), and the spectral gate folds into the
    projection weights for the dwconv(y) branch.
"""

import numpy as np
from contextlib import ExitStack

import concourse.bass as bass
import concourse.bacc as bacc
import concourse.tile as tile
from concourse import mybir
from concourse.bass_utils import run_bass_kernel_spmd

FP32 = mybir.dt.float32
BF16 = mybir.dt.bfloat16
AF = mybir.ActivationFunctionType
ALU = mybir.AluOpType

B, C, H, W = 2, 128, 256, 256
HD, DH = 8, 16
NCORES = 8
RPC = H // NCORES            # 32 rows per core
HH, WW = RPC + 2, W + 2      # 34 x 258 halo'd band
FREE = HH * WW               # 8772
NLOC = RPC * W               # 8192 output positions per band per batch
NCH_T = NLOC // 128          # 64 transposed chunks
NCH_A = NLOC // 512          # 16 layout-A chunks
NTOT = float(H * W)          # global spatial size


def _emit(tc, io):
    nc = tc.nc
    ctx = ExitStack()

    wpool = ctx.enter_context(tc.tile_pool(name="wpool", bufs=1))
    xpool = ctx.enter_context(tc.tile_pool(name="xpool", bufs=2))
    ypool = ctx.enter_context(tc.tile_pool(name="ypool", bufs=2))
    yspool = ctx.enter_context(tc.tile_pool(name="yspool", bufs=2))
    vpool = ctx.enter_context(tc.tile_pool(name="vpool", bufs=2))
    y2pool = ctx.enter_context(tc.tile_pool(name="y2pool", bufs=2))
    spool = ctx.enter_context(tc.tile_pool(name="spool", bufs=1))
    rpool = ctx.enter_context(tc.tile_pool(name="rpool", bufs=6))
    mpool = ctx.enter_context(tc.tile_pool(name="mpool", bufs=2))
    opool = ctx.enter_context(tc.tile_pool(name="opool", bufs=3))
    psA = ctx.enter_context(tc.tile_pool(name="psA", bufs=2, space="PSUM"))
    psQK = ctx.enter_context(tc.tile_pool(name="psQK", bufs=3, space="PSUM"))
    psG = ctx.enter_context(tc.tile_pool(name="psG", bufs=1, space="PSUM"))
    dpool = ctx.enter_context(tc.tile_pool(name="dram", bufs=1, space="DRAM"))

    # ---- load weights/constants ----
    def wload(name, shape, dt=BF16, eng=None):
        t = wpool.tile(shape, dt, tag=name)
        (eng or nc.sync).dma_start(out=t[:], in_=io[name][:])
        return t

    saw1t = wload("saw1t", [128, 32])
    w2rep = wload("w2rep", [128, 32])
    w3rep = wload("w3rep", [128, 1])
    spw1t = wload("spw1t", [128, 16], FP32)
    spw2t = wload("spw2t", [16, 16], FP32)
    spw3t = wload("spw3t", [16, 128], FP32)
    projt = wload("projt", [128, 128], FP32)
    wydc = wload("wydc", [128, 9], FP32)
    consts = wload("consts", [128, 386], FP32)
    eye = consts[:, 0:128]
    bdmask = consts[:, 128:256]
    tempp = consts[:, 256:257]
    onesrow = consts[0:1, 257:385]

    # ---- input DMAs spread over the 3 dynamic queues; critical tiles first ----
    # sync:   small weights -> x0 -> w9v;  gpsimd: y0a -> y1;
    # scalar: y0b -> w9qk -> x1.  ysh derived on-chip (scalar b0 / gpsimd b1).
    xts, yts, yshs, vts, y2ts, saTs = [], [], [], [], [], []
    SPLIT = 17 * WW
    yt0 = ypool.tile([128, FREE], BF16, tag="y", name="yt0")
    yts.append(yt0)
    y2d0 = io["yh"][0].rearrange("c h w -> c (h w)")
    nc.gpsimd.dma_start(out=yt0[:, 0:SPLIT], in_=y2d0[:, 0:SPLIT])
    nc.scalar.dma_start(out=yt0[:, SPLIT:FREE], in_=y2d0[:, SPLIT:FREE])
    xt0 = xpool.tile([128, FREE], BF16, tag="x", name="xt0")
    xts.append(xt0)
    nc.sync.dma_start(out=xt0[:], in_=io["xh"][0].rearrange("c h w -> c (h w)"))
    w9qk = wload("w9qk", [128, 9 * 256], eng=nc.scalar)
    yt1 = ypool.tile([128, FREE], BF16, tag="y", name="yt1")
    yts.append(yt1)
    nc.gpsimd.dma_start(out=yt1[:], in_=io["yh"][1].rearrange("c h w -> c (h w)"))
    xt1 = xpool.tile([128, FREE], BF16, tag="x", name="xt1")
    xts.append(xt1)
    nc.scalar.dma_start(out=xt1[:], in_=io["xh"][1].rearrange("c h w -> c (h w)"))
    w9v = wload("w9v", [128, 9 * 128], eng=nc.sync)
    # ysh[b] = y[b] shifted left by one column (for 4B-aligned odd-column taps)
    for b in range(B):
        ysh = yspool.tile([128, FREE], BF16, tag="ysh")
        nc.scalar.copy(ysh[:, 0:FREE - 1], yts[b][:, 1:FREE])
        yshs.append(ysh)

    arst1 = mpool.tile([128, 262], FP32, tag="arst1")

    # ---- y depthwise conv steps (vector + gpsimd engines, interleaved) ----
    # every GP_PERIOD-th chunk runs on gpsimd, the rest on vector; taps are
    # reordered so the ysh-dependent (odd-column) taps come last per chunk.
    GP_PERIOD = 999  # gpsimd lacks TensorScalarPtr; keep ydw on vector
    TAP_ORDER = [0, 2, 3, 5, 6, 8, 1, 4, 7]
    ydw_steps = []

    def make_ydw(b):
        yt, ysh = yts[b], yshs[b]
        y2t = y2pool.tile([128, NLOC], BF16, tag="y2")
        y2ts.append(y2t)
        for nn in range(NCH_A):
            r0 = 2 * nn
            eng = nc.gpsimd if nn % GP_PERIOD == GP_PERIOD - 1 else nc.vector
            ov = y2t[:, 512 * nn:512 * nn + 512].rearrange(
                "p (r w) -> p r w", r=2)
            for ii, t in enumerate(TAP_ORDER):
                ti, tj = t // 3, t % 3
                src, tjj = (ysh, 0) if tj == 1 else (yt, tj)
                iv = src[:].rearrange("p (h w) -> p h w", h=HH)[
                    :, r0 + ti:r0 + ti + 2, tjj:tjj + 256]
                wc = wydc[:, t:t + 1]
                if ii == 0:
                    ydw_steps.append(
                        lambda eng=eng, ov=ov, iv=iv, wc=wc:
                        eng.tensor_scalar_mul(ov, iv, wc))
                else:
                    ydw_steps.append(
                        lambda eng=eng, ov=ov, iv=iv, wc=wc:
                        eng.scalar_tensor_tensor(
                            ov, iv, wc, ov, ALU.mult, ALU.add))

    def pop_ydw(n):
        for _ in range(n):
            if ydw_steps:
                ydw_steps.pop(0)()

    # ================= per-batch: sa gate + qk conv/gram =================
    for b in range(B):
        cb = 131 * b
        xt, yt = xts[b], yts[b]
        make_ydw(b)

        # ---- spatial-attention gate: sa = sigmoid(w3 relu(w2 relu(w1 y))) ----
        s1 = spool.tile([128, 2048], BF16, tag="s1")
        s2 = spool.tile([128, 2048], BF16, tag="s2")
        for g in range(4):
            ps1 = psA.tile([128, 512], FP32, tag="a")
            for k in range(4):
                nn = 4 * g + k
                r0 = 2 * nn
                yv = yt[:].rearrange("p (h w) -> p h w", h=HH)[
                    :, r0 + 1:r0 + 3, 1:257]
                nc.tensor.matmul(ps1[32 * k:32 * k + 32, :], saw1t[:, :], yv,
                                 start=True, stop=True, tile_position=(0, 32 * k))
            if g % 2 == 0:
                nc.vector.tensor_scalar_max(s1[:, 512 * g:512 * g + 512], ps1[:, :], 0.0)
            else:
                nc.scalar.activation(s1[:, 512 * g:512 * g + 512], ps1[:, :], AF.Relu)
        for g in range(4):
            ps2 = psA.tile([128, 512], FP32, tag="a")
            for k in range(4):
                nc.tensor.matmul(ps2[32 * k:32 * k + 32, :],
                                 w2rep[32 * k:32 * k + 16, :],
                                 s1[32 * k:32 * k + 16, 512 * g:512 * g + 512],
                                 start=True, stop=True,
                                 tile_position=(32 * k, 32 * k))
            if g % 2 == 0:
                nc.vector.tensor_scalar_max(s2[:, 512 * g:512 * g + 512], ps2[:, :], 0.0)
            else:
                nc.scalar.activation(s2[:, 512 * g:512 * g + 512], ps2[:, :], AF.Relu)
        # stage 3: saT[n] packed as [128, 64] (col j holds n = 128j + p)
        sv_ps = psG.tile([128, 66], FP32, tag="sv")
        for j in range(NCH_T):
            nn, off = j // 4, (j % 4) * 128
            g, k = nn // 4, nn % 4
            nc.tensor.matmul(sv_ps[:, j:j + 1],
                             s2[32 * k:32 * k + 16,
                                512 * g + off:512 * g + off + 128],
                             w3rep[32 * k:32 * k + 16, :],
                             start=True, stop=True, tile_position=(32 * k, 0))
        saT = mpool.tile([128, 64], FP32, tag="saT")
        nc.scalar.activation(saT[:], sv_ps[:, 0:64], AF.Sigmoid)
        saTs.append(saT)

        # ---- v-mean from x window sums (replaces accumulating over v) ----
        # S_t[i] = sum over rows [ti,ti+32) cols [tj,tj+256) of the halo'd x;
        # vmean_part = sum_t W_t^T S_t, AllReduced along with the grams.
        x3 = xt[:].rearrange("p (h w) -> p h w", h=HH)
        rsum = mpool.tile([128, HH], FP32, tag="rsum")       # per-row sums, cols 0..255
        nc.vector.tensor_reduce(rsum[:], x3[:, :, 0:256], mybir.AxisListType.X,
                                ALU.add)
        bsum = mpool.tile([128, 3], FP32, tag="bsum")        # 32-row window sums
        for ti in range(3):
            nc.vector.tensor_reduce(bsum[:, ti:ti + 1], rsum[:, ti:ti + 32],
                                    mybir.AxisListType.X, ALU.add)
        x3w = xt[:].rearrange("p (h w) -> p w h", h=HH)      # [128, 258, 34]
        csum = mpool.tile([128, 12], FP32, tag="csum")       # col strips w=0,1,256,257
        for wi, w in enumerate((0, 1, 256, 257)):
            for ti in range(3):
                nc.vector.tensor_reduce(csum[:, 3 * wi + ti:3 * wi + ti + 1],
                                        x3w[:, w:w + 1, ti:ti + 32],
                                        mybir.AxisListType.X, ALU.add)
        stap = mpool.tile([128, 9], FP32, tag="stap")
        for ti in range(3):
            nc.vector.tensor_copy(stap[:, 3 * ti:3 * ti + 1], bsum[:, ti:ti + 1])
            # tj=1: - col0 + col256 ; tj=2: also - col1 + col257
            nc.vector.tensor_tensor(stap[:, 3 * ti + 1:3 * ti + 2],
                                    stap[:, 3 * ti:3 * ti + 1],
                                    csum[:, ti:ti + 1], ALU.subtract)
            nc.vector.tensor_tensor(stap[:, 3 * ti + 1:3 * ti + 2],
                                    stap[:, 3 * ti + 1:3 * ti + 2],
                                    csum[:, 6 + ti:7 + ti], ALU.add)
            nc.vector.tensor_tensor(stap[:, 3 * ti + 2:3 * ti + 3],
                                    stap[:, 3 * ti + 1:3 * ti + 2],
                                    csum[:, 3 + ti:4 + ti], ALU.subtract)
            nc.vector.tensor_tensor(stap[:, 3 * ti + 2:3 * ti + 3],
                                    stap[:, 3 * ti + 2:3 * ti + 3],
                                    csum[:, 9 + ti:10 + ti], ALU.add)
        stapb = mpool.tile([128, 9], BF16, tag="stapb")
        nc.vector.tensor_copy(stapb[:], stap[:])
        for t in range(9):
            nc.tensor.matmul(sv_ps[:, 64:65], w9v[:, 128 * t:128 * t + 128],
                             stapb[:, t:t + 1], start=(t == 0), stop=(t == 8))
        nc.vector.tensor_copy(arst1[:, cb + 130:cb + 131], sv_ps[:, 64:65])

        # ---- qk conv (transposed layout) + gram accumulation ----
        Gt = psG.tile([128, 384], FP32, tag="G")
        for j in range(NCH_T):
            r, c0 = j // 2, (j % 2) * 128
            pqk = psQK.tile([128, 256], FP32, tag="qk")
            for t in range(9):
                ti, tj = t // 3, t % 3
                base = (r + ti) * WW + c0 + tj
                nc.tensor.matmul(pqk[:, :], xt[:, base:base + 128],
                                 w9qk[:, 256 * t:256 * t + 256],
                                 start=(t == 0), stop=(t == 8))
            rt = rpool.tile([128, 256], BF16, tag="ring")
            # q scaled by sa (per-partition in transposed layout), k plain
            nc.scalar.activation(rt[:, 0:128], pqk[:, 0:128], AF.Copy,
                                 scale=saT[:, j:j + 1])
            nc.vector.tensor_copy(rt[:, 128:256], pqk[:, 128:256])
            nc.tensor.matmul(Gt[:, 0:256], rt[:, 0:128], rt[:, 0:256],
                             start=(j == 0), stop=(j == NCH_T - 1),
                             skip_group_check=True)
            nc.tensor.matmul(Gt[:, 256:384], rt[:, 128:256], rt[:, 128:256],
                             start=(j == 0), stop=(j == NCH_T - 1),
                             skip_group_check=True)
            pop_ydw(2)

        # ---- stats staging for AllReduce 1 ----
        junk = mpool.tile([128, 128], FP32, tag="junk")
        nc.vector.tensor_copy(arst1[:, cb:cb + 128], Gt[:, 128:256])  # Gqk
        nc.vector.scalar_tensor_tensor(junk[:], Gt[:, 0:128], 1.0, eye,
                                       ALU.mult, ALU.mult,
                                       accum_out=arst1[:, cb + 128:cb + 129])
        nc.vector.scalar_tensor_tensor(junk[:], Gt[:, 256:384], 1.0, eye,
                                       ALU.mult, ALU.mult,
                                       accum_out=arst1[:, cb + 129:cb + 130])

    # ================= AllReduce: grams + norms + v means =================
    din1 = dpool.tile([128, 262], FP32, tag="din1")
    dout1 = dpool.tile([128, 262], FP32, tag="dout1")
    nc.gpsimd.dma_start(out=din1[:], in_=arst1[:])
    nc.gpsimd.collective_compute(
        "AllReduce", ALU.add,
        replica_groups=[list(range(NCORES))],
        ins=[din1[:].opt()], outs=[dout1[:].opt()])
    arres1 = mpool.tile([128, 262], FP32, tag="arres1")
    nc.gpsimd.dma_start(out=arres1[:], in_=dout1[:])

    # ================= v convs (hide the AllReduce) =================
    for b in range(B):
        xt = xts[b]
        vt = vpool.tile([128, NLOC], BF16, tag="v")
        vts.append(vt)
        for nn in range(NCH_A):
            r0 = 2 * nn
            pv = psA.tile([128, 512], FP32, tag="a")
            for t in range(9):
                ti, tj = t // 3, t % 3
                xv = xt[:].rearrange("p (h w) -> p h w", h=HH)[
                    :, r0 + ti:r0 + ti + 2, tj:tj + 256]
                nc.tensor.matmul(pv[:, :], w9v[:, 128 * t:128 * t + 128], xv,
                                 start=(t == 0), stop=(t == 8))
            nc.scalar.copy(vt[:, 512 * nn:512 * nn + 512], pv[:, :])
            pop_ydw(1)
    pop_ydw(len(ydw_steps))

    # ================= post-AllReduce attention math =================
    meffts, attnts = [], []
    for b in range(B):
        cb = 131 * b
        # 1/max(sqrt(d), eps) with one Newton-rsqrt refinement
        rqk = mpool.tile([128, 2], FP32, tag="rqk")
        srt = mpool.tile([128, 2], FP32, tag="srt")
        dcat = arres1[:, cb + 128:cb + 130]  # [qd kd]
        nc.scalar.activation(srt[:], dcat, AF.Sqrt)
        nc.vector.tensor_scalar_max(srt[:], srt[:], 1e-12)
        nc.vector.reciprocal(rqk[:], srt[:])
        r2 = mpool.tile([128, 2], FP32, tag="r2")
        nc.vector.tensor_tensor(r2[:], rqk[:], rqk[:], ALU.mult)
        nc.vector.tensor_tensor(r2[:], r2[:], dcat, ALU.mult)
        nc.vector.tensor_scalar(r2[:], r2[:], -0.5, 1.5, ALU.mult, ALU.add)
        nc.vector.tensor_tensor(rqk[:], rqk[:], r2[:], ALU.mult)
        rqt = mpool.tile([128, 1], FP32, tag="rqt")
        nc.vector.tensor_tensor(rqt[:], rqk[:, 0:1], tempp, ALU.mult)

        # broadcast rk along partitions: rkb[p, d] = rk[d]
        ps1 = psA.tile([128, 128], FP32, tag="a")
        nc.tensor.matmul(ps1[0:1, :], rqk[:, 1:2], eye, start=True, stop=True)
        rkrow = mpool.tile([1, 128], FP32, tag="rkrow")
        nc.scalar.copy(rkrow[:], ps1[0:1, :])
        ps2 = psA.tile([128, 128], FP32, tag="a")
        nc.tensor.matmul(ps2[:, :], onesrow, rkrow[:], start=True, stop=True)

        # logits -> masked softmax -> attnBD
        gh = mpool.tile([128, 128], FP32, tag="gh")
        nc.vector.scalar_tensor_tensor(gh[:], arres1[:, cb:cb + 128], rqt[:, 0:1],
                                       ps2[:, :], ALU.mult, ALU.mult)
        sm = mpool.tile([128, 128], FP32, tag="sm")
        nc.scalar.activation(sm[:], gh[:], AF.Exp)
        rs = mpool.tile([128, 1], FP32, tag="rs")
        nc.vector.scalar_tensor_tensor(sm[:], sm[:], 1.0, bdmask,
                                       ALU.mult, ALU.mult, accum_out=rs[:])
        nc.vector.reciprocal(rs[:], rs[:])
        attn = mpool.tile([128, 128], FP32, tag="attn")
        nc.vector.tensor_scalar_mul(attn[:], sm[:], rs[:, 0:1])

        # MeffT = (proj @ attnBD)^T: lhsT=attn, rhs=projT
        psM = psA.tile([128, 128], FP32, tag="a")
        nc.tensor.matmul(psM[:, :], attn[:], projt[:], start=True, stop=True)
        mefft = mpool.tile([128, 128], BF16, tag="mefft")
        nc.scalar.copy(mefft[:], psM[:, :])
        meffts.append(mefft)

        # attn^T (for pooled = attnBD @ v_mean)
        psT = psA.tile([128, 128], FP32, tag="a")
        nc.tensor.transpose(psT[:, :], attn[:], eye)
        attnt = mpool.tile([128, 128], FP32, tag="attnt")
        nc.vector.tensor_copy(attnt[:], psT[:, :])
        attnts.append(attnt)

    # ============ spectral gate + final projection ============
    for b in range(B):
        cb = 131 * b
        psP = psA.tile([128, 1], FP32, tag="a")
        nc.tensor.matmul(psP[:, :], attnts[b][:], arres1[:, cb + 130:cb + 131],
                         start=True, stop=True)
        pooled = mpool.tile([128, 1], FP32, tag="pooled")
        nc.scalar.activation(pooled[:], psP[:, :], AF.Copy, scale=1.0 / NTOT)

        # spectral gate MLP: sigmoid(w3 gelu(w2 gelu(w1 pooled)))
        psg1 = psA.tile([16, 1], FP32, tag="a")
        nc.tensor.matmul(psg1[:, :], spw1t[:], pooled[:], start=True, stop=True)
        g1 = mpool.tile([16, 1], FP32, tag="g1")
        nc.scalar.activation(g1[:], psg1[:, :], AF.Gelu)
        psg2 = psA.tile([16, 1], FP32, tag="a")
        nc.tensor.matmul(psg2[:, :], spw2t[:], g1[:], start=True, stop=True)
        g2 = mpool.tile([16, 1], FP32, tag="g2")
        nc.scalar.activation(g2[:], psg2[:, :], AF.Gelu)
        psg3 = psA.tile([128, 1], FP32, tag="a")
        nc.tensor.matmul(psg3[:, :], spw3t[:], g2[:], start=True, stop=True)
        spec = mpool.tile([128, 1], FP32, tag="spec")
        nc.scalar.activation(spec[:], psg3[:, :], AF.Sigmoid)

        # fold spectral gate into the projection of the dwconv(y) branch
        p2t = mpool.tile([128, 128], BF16, tag="p2t")
        nc.vector.tensor_scalar_mul(p2t[:], projt[:], spec[:, 0:1])

        # final fused projection
        out2d = io["out"][b].rearrange("c h w -> c (h w)")
        for nn in range(NCH_A):
            pf = psA.tile([128, 512], FP32, tag="a")
            nc.tensor.matmul(pf[:, :], meffts[b][:],
                             vts[b][:, 512 * nn:512 * nn + 512],
                             start=True, stop=False)
            nc.tensor.matmul(pf[:, :], p2t[:],
                             y2ts[b][:, 512 * nn:512 * nn + 512],
                             start=False, stop=True)
            ot = opool.tile([128, 512], FP32, tag="ot")
            if nn % 2 == 0:
                nc.scalar.copy(ot[:], pf[:, :])
                nc.sync.dma_start(out=out2d[:, 512 * nn:512 * nn + 512], in_=ot[:])
            else:
                nc.vector.tensor_copy(ot[:], pf[:, :])
                nc.gpsimd.dma_start(out=out2d[:, 512 * nn:512 * nn + 512], in_=ot[:])

    ctx.close()


def build_nc():
    nc = bacc.Bacc("TRN2", target_bir_lowering=False, debug=False,
                   num_devices=NCORES)
    io = {}

    def inp(name, shape, dt):
        io[name] = nc.dram_tensor(name, shape, dt, kind="ExternalInput")

    inp("xh", [B, C, HH, WW], BF16)
    inp("yh", [B, C, HH, WW], BF16)
    inp("w9qk", [128, 9 * 256], BF16)
    inp("w9v", [128, 9 * 128], BF16)
    inp("saw1t", [128, 32], BF16)
    inp("w2rep", [128, 32], BF16)
    inp("w3rep", [128, 1], BF16)
    inp("spw1t", [128, 16], FP32)
    inp("spw2t", [16, 16], FP32)
    inp("spw3t", [16, 128], FP32)
    inp("projt", [128, 128], FP32)
    inp("wydc", [128, 9], FP32)
    inp("consts", [128, 386], FP32)
    io["out"] = nc.dram_tensor("out", [B, C, RPC, W], FP32, kind="ExternalOutput")

    with tile.TileContext(nc) as tc:
        _emit(tc, io)
    nc.finalize()
    return nc


_CACHE = {}


def _prep_host(x, y, qkv_w, qkv_dw_w, proj_w, sa_w1, sa_w2, sa_w3,
               sp_w1, sp_w2, sp_w3, dw_w, temperature):
    import ml_dtypes
    bf = ml_dtypes.bfloat16
    f32 = np.float32

    x = np.asarray(x, f32)
    y = np.asarray(y, f32)
    xp = np.zeros((B, C, H + 2, W + 2), f32)
    xp[:, :, 1:H + 1, 1:W + 1] = x
    yp = np.zeros((B, C, H + 2, W + 2), f32)
    yp[:, :, 1:H + 1, 1:W + 1] = y
    xp = xp.astype(bf)
    yp = yp.astype(bf)

    qkv_w = np.asarray(qkv_w, f32)
    dw = np.asarray(qkv_dw_w, f32).reshape(3 * C, 9)
    w9qk = np.concatenate(
        [(qkv_w[:256] * dw[:256, t:t + 1]).T for t in range(9)], axis=1)  # [128, 9*256]
    w9v = np.concatenate(
        [(qkv_w[256:] * dw[256:, t:t + 1]).T for t in range(9)], axis=1)  # [128, 9*128]
    wydc = np.asarray(dw_w, f32).reshape(C, 9)                            # [128, 9]

    w2rep = np.zeros((128, 32), f32)
    w3rep = np.zeros((128, 1), f32)
    for k in range(4):
        w2rep[32 * k:32 * k + 16, 0:16] = np.asarray(sa_w2, f32).T
        w3rep[32 * k:32 * k + 16] = np.asarray(sa_w3, f32).T
    saw1tp = np.zeros((128, 32), f32)
    saw1tp[:, 0:16] = np.asarray(sa_w1, f32).T

    consts = np.zeros((128, 386), f32)
    consts[:, 0:128] = np.eye(128, dtype=f32)
    ci = np.arange(128) // DH
    consts[:, 128:256] = (ci[:, None] == ci[None, :]).astype(f32)
    consts[:, 256] = np.asarray(temperature, f32).reshape(HD)[ci]
    consts[0, 257:385] = 1.0

    common = {
        "w9qk": w9qk.astype(bf), "w9v": w9v.astype(bf),
        "saw1t": saw1tp.astype(bf),
        "w2rep": w2rep.astype(bf), "w3rep": w3rep.astype(bf),
        "spw1t": np.asarray(sp_w1, f32).T.copy(),
        "spw2t": np.asarray(sp_w2, f32).T.copy(),
        "spw3t": np.asarray(sp_w3, f32).T.copy(),
        "projt": np.asarray(proj_w, f32).T.copy(),
        "wydc": wydc,
        "consts": consts,
    }
    in_maps = []
    for i in range(NCORES):
        m = dict(common)
        m["xh"] = np.ascontiguousarray(xp[:, :, 32 * i:32 * i + HH, :])
        m["yh"] = np.ascontiguousarray(yp[:, :, 32 * i:32 * i + HH, :])
        in_maps.append(m)
    return in_maps


def kernel(**inputs):
    if "nc" not in _CACHE:
        _CACHE["nc"] = build_nc()
    nc = _CACHE["nc"]
    in_maps = _prep_host(**inputs)
    res = run_bass_kernel_spmd(nc, in_maps, core_ids=list(range(NCORES)))
    shards = [res.results[i]["out"] for i in range(NCORES)]
    return np.concatenate(shards, axis=2).astype(np.float32)
